# revision 1
# baseline (speedup 1.0000x reference)
"""LCAConv (locally competitive algorithm, convolutional sparse coding) on
8 trn2 NeuronCores — data-parallel over batch (1 sample per core).

Math (matches the jax reference):
  x2   = standardize(x)                       per-sample, ddof=1, eps 1e-12
  b    = conv(x2, D, pad=3)                   input drive [32,64,64]
  G    = conv(D, D, pad=6)                    Gram tensor [32,32,13,13]
  u_1  = 0.01*b;  a_t = soft_threshold(u_{t-1}, 0.1)
  u_t  = u_{t-1} + 0.01*b - 0.01*clip(u_{t-1}, +-0.1) - 0.01*conv(a_t, G, pad=6)
  out  = a_10 = ST(u_9)   ->  8 Gram-conv iterations on device.

Device mapping: 2x2 phase-packed layout. Pixel (y,x) = (2jy+sy, 2jx+sx).
Activations live as [128, 38, 38]: partition (sy*2+sx)*32 + c, padded j-grid
(pad 3). Each conv tap-group (dy,dx) in [-3,3]^2 is one [K=128 -> M=128]
matmul over N=512-column chunks with a host-packed banded lhsT; u is
accumulated and kept resident in PSUM across all iterations. Matmuls run in
float32r (1 col/cycle, ~1e-4 operand rounding; final error ~1e-5).
"""
import os
import sys
import types
import numpy as np

# ---------------------------------------------------------------- constants
NN, IC, KH, KW = 32, 3, 7, 7          # neurons, in-channels, kernel
H = W = 64
J = 32                                 # phase-grid extent (64/2)
PAD = 3                                # j-space padding
JP = J + 2 * PAD                       # padded j-grid: 38
THRESH, TAU, LCA_ITERS = 0.1, 100.0, 10
ITERS_DEV = LCA_ITERS - 2              # first iter folded into b, last is ST only
NCHUNK = 2                             # psum column chunks (jy halves)
CH = J // NCHUNK                       # 16 jy rows per chunk
B = 8                                  # batch == n cores

_CACHE = {}
_ZEROS = np.zeros((128, JP, JP), np.float32)


def _phase_pack(img):
    """[C,64,64] -> [(s*C+..) wait: row (sy*2+sx)*C + c is for acts; x uses
    c*4 + (sy*2+sx). This helper returns [C,2,2,32,32] = img[c, 2jy+sy, 2jx+sx]
    indexed [c, sy, sx, jy, jx]."""
    C = img.shape[0]
    return img.reshape(C, J, 2, J, 2).transpose(0, 2, 4, 1, 3)


def _host_pack(D):
    """Everything derived from D only (weights): G, banded lhsT tiles."""
    D2 = np.asarray(D, np.float64).reshape(NN, IC, KH, KW)

    # Gram tensor G[n,m,py,px] = sum_{c,i,j} D2[n,c,i,j] * D2[m,c,i+py-6,j+px-6]
    Dp = np.zeros((NN, IC, KH + 12, KW + 12))
    Dp[:, :, 6:6 + KH, 6:6 + KW] = D2
    win = np.lib.stride_tricks.sliding_window_view(Dp, (KH, KW), axis=(2, 3))
    # win[m,c,py,px,i,j] = Dp[m,c,py+i,px+j]
    G = np.einsum('ncij,mcpqij->mnpq', D2, win, optimize=True)  # [32,32,13,13]

    # G-conv lhsT: 49 tiles, GPK[i, (t)*32+cin, (s)*32+cout] = -0.01*G[cout,cin,ky,kx]
    GPK = np.zeros((49, 128, 128), np.float32)
    for dy in range(-3, 4):
        for dx in range(-3, 4):
            i = (dy + 3) * 7 + (dx + 3)
            for ty in range(2):
                for tx in range(2):
                    for sy in range(2):
                        for sx in range(2):
                            ky = 2 * dy + ty - sy + 6
                            kx = 2 * dx + tx - sx + 6
                            if 0 <= ky <= 12 and 0 <= kx <= 12:
                                t = ty * 2 + tx
                                s = sy * 2 + sx
                                GPK[i, t * 32:(t + 1) * 32, s * 32:(s + 1) * 32] = \
                                    (-0.01 * G[:, :, ky, kx].T).astype(np.float32)

    # b-conv lhsT: 25 tiles, DPK[i, c*4+t, s*32+cout] = 0.01*D2[cout,c,ky,kx]
    DPK = np.zeros((25, 12, 128), np.float32)
    for dy in range(-2, 3):
        for dx in range(-2, 3):
            i = (dy + 2) * 5 + (dx + 2)
            for ty in range(2):
                for tx in range(2):
                    for sy in range(2):
                        for sx in range(2):
                            ky = 2 * dy + ty - sy + PAD
                            kx = 2 * dx + tx - sx + PAD
                            if 0 <= ky <= 6 and 0 <= kx <= 6:
                                t = ty * 2 + tx
                                s = sy * 2 + sx
                                for c in range(IC):
                                    DPK[i, c * 4 + t, s * 32:(s + 1) * 32] = \
                                        (0.01 * D2[:, c, ky, kx]).astype(np.float32)

    IPK = np.concatenate(
        [np.eye(128, dtype=np.float32), -0.01 * np.eye(128, dtype=np.float32)],
        axis=1)  # [128, 256]

    # DMA-friendly partition-major layouts
    return {
        "GPK": np.ascontiguousarray(GPK.transpose(1, 0, 2)),   # [128, 49, 128]
        "DPK": np.ascontiguousarray(DPK.transpose(1, 0, 2)),   # [12, 25, 128]
        "IPK": IPK,
    }


def _pack_x(xs):
    """[3,64,64] raw x -> [12, 38, 38] phase layout, zero padded."""
    P = _phase_pack(np.asarray(xs, np.float32))          # [3,2,2,32,32]
    out = np.zeros((12, JP, JP), np.float32)
    out[:, PAD:PAD + J, PAD:PAD + J] = P.reshape(12, J, J)
    return out


def _unpack_a(res):
    """[128, 32, 32] phase layout -> [32, 64, 64]."""
    r = res.reshape(2, 2, NN, J, J)                      # [sy,sx,c,jy,jx]
    a = np.empty((NN, H, W), np.float32)
    av = a.reshape(NN, J, 2, J, 2)
    av[...] = r.transpose(2, 3, 0, 4, 1)                 # [c,jy,sy,jx,sx]
    return a


# ------------------------------------------------------------- device build
def _install_ntff_hook():
    """Re-register the NTFF profile hook this image's antenv lacks."""
    try:
        from antenv.axon_hooks import get_axon_ntff_profile_hook  # noqa: F401
        return
    except ImportError:
        pass
    try:
        import antenv
        mod = types.ModuleType("antenv.axon_hooks")
        _h = [None]
        mod.set_axon_ntff_profile_hook = lambda h: _h.__setitem__(0, h)
        mod.get_axon_ntff_profile_hook = lambda: _h[0]
        sys.modules["antenv.axon_hooks"] = mod
        antenv.axon_hooks = mod
        if "/root/.axon_site" not in sys.path:
            sys.path.insert(0, "/root/.axon_site")
        from trn_agent_boot.trn_boot import _ntff_profile_via_ctypes
        hook = _ntff_profile_via_ctypes('/opt/axon/libaxon_pjrt.so')
        if hook is not None:
            mod.set_axon_ntff_profile_hook(hook)
    except Exception:
        pass


def _build(iters_dev=ITERS_DEV):
    import concourse.tile as tile
    from concourse import bacc, mybir

    f32 = mybir.dt.float32
    f32r = mybir.dt.float32r
    r = lambda ap: ap.bitcast(f32r)

    nc = bacc.Bacc(None)
    XL2 = nc.declare_dram_parameter("XL2", [12, JP, JP], f32, isOutput=False)
    GPK = nc.declare_dram_parameter("GPK", [128, 49, 128], f32r, isOutput=False)
    DPK = nc.declare_dram_parameter("DPK", [12, 25, 128], f32r, isOutput=False)
    IPK = nc.declare_dram_parameter("IPK", [128, 256], f32r, isOutput=False)
    ZR = nc.declare_dram_parameter("ZR", [128, JP, JP], f32r, isOutput=False)
    ONESD = nc.declare_dram_parameter("ONESD", [12, 1], f32r, isOutput=False)
    AOUT = nc.declare_dram_parameter("AOUT", [128, J, J], f32, isOutput=True)

    with tile.TileContext(nc) as tc:
        import contextlib
        with contextlib.ExitStack() as ctx:
            sb = ctx.enter_context(tc.tile_pool(name="sb", bufs=1))
            ps = ctx.enter_context(tc.tile_pool(name="ps", bufs=1, space="PSUM"))

            # ---- constants / inputs into SBUF
            gt = sb.tile([128, 49, 128], f32r, tag="gt", name="gt")
            dt_ = sb.tile([12, 25, 128], f32r, tag="dt", name="dt")
            ipk = sb.tile([128, 256], f32r, tag="ipk", name="ipk")
            X = sb.tile([12, JP, JP], f32, tag="X", name="X")
            Xr = sb.tile([12, JP, JP], f32r, tag="Xr", name="Xr")
            nc.sync.dma_start(out=gt[:], in_=GPK[:])
            nc.sync.dma_start(out=dt_[:], in_=DPK[:])
            nc.sync.dma_start(out=ipk[:], in_=IPK[:])
            nc.sync.dma_start(out=X[:], in_=XL2[:])

            Xi = X[:, PAD:PAD + J, PAD:PAD + J]          # [12,32,32] interior
            Xri = Xr[:, PAD:PAD + J, PAD:PAD + J]

            # ---- standardization: mean/rstd over the 12288 real values
            nc.sync.dma_start(out=Xr[:], in_=ZR[0:12, :, :])
            nc.vector.tensor_copy(Xri, Xi)               # f32 -> f32r round
            ones = sb.tile([12, 1], f32r, tag="ones", name="ones")
            nc.sync.dma_start(out=ones[:], in_=ONESD[:])
            sq = sb.tile([12, J, J], f32r, tag="sq", name="sq")
            nc.vector.tensor_mul(sq[:], Xi, Xi)
            psx = [ps.tile([1, 512], f32, tag="psx", name=f"psx{k}") for k in range(2)]
            psq = [ps.tile([1, 512], f32, tag="psq", name=f"psq{k}") for k in range(2)]
            for k in range(2):
                nc.tensor.matmul(psx[k][:], ones[:],
                                 Xri[:, k * CH:(k + 1) * CH, :],
                                 start=True, stop=True)
                nc.tensor.matmul(psq[k][:], ones[:],
                                 sq[:, k * CH:(k + 1) * CH, :],
                                 start=True, stop=True)
            sc = sb.tile([1, 8], f32, tag="sc", name="sc")          # scratch scalars
            nc.vector.reduce_sum(sc[:, 0:1], psx[0][:], axis=mybir.AxisListType.X)
            nc.vector.reduce_sum(sc[:, 1:2], psx[1][:], axis=mybir.AxisListType.X)
            nc.vector.reduce_sum(sc[:, 2:3], psq[0][:], axis=mybir.AxisListType.X)
            nc.vector.reduce_sum(sc[:, 3:4], psq[1][:], axis=mybir.AxisListType.X)
            n = float(IC * H * W)
            nc.vector.tensor_add(sc[:, 0:1], sc[:, 0:1], sc[:, 1:2])   # Sx
            nc.vector.tensor_add(sc[:, 2:3], sc[:, 2:3], sc[:, 3:4])   # Sxx
            # var = (Sxx - Sx^2/n) / (n-1)
            nc.vector.tensor_mul(sc[:, 4:5], sc[:, 0:1], sc[:, 0:1])   # Sx^2
            nc.vector.tensor_scalar_mul(sc[:, 4:5], sc[:, 4:5], 1.0 / n)
            nc.vector.tensor_sub(sc[:, 4:5], sc[:, 2:3], sc[:, 4:5])
            nc.vector.tensor_scalar_mul(sc[:, 4:5], sc[:, 4:5], 1.0 / (n - 1.0))
            nc.scalar.activation(sc[:, 4:5], sc[:, 4:5],
                                 mybir.ActivationFunctionType.Sqrt)
            nc.vector.tensor_scalar_add(sc[:, 4:5], sc[:, 4:5], 1e-12)
            nc.vector.reciprocal(sc[:, 4:5], sc[:, 4:5])               # rstd
            nc.vector.tensor_scalar_mul(sc[:, 0:1], sc[:, 0:1], 1.0 / n)  # mean
            # broadcast mean/rstd to the 12 partitions via a DRAM bounce
            nc.vector.tensor_copy(sc[:, 1:2], sc[:, 4:5])
            scd = nc.dram_tensor("scd", [1, 2], f32)
            nc.sync.dma_start(out=scd[:], in_=sc[:, 0:2])
            ms = sb.tile([12, 2], f32, tag="ms", name="ms")
            nc.sync.dma_start(out=ms[:], in_=scd[0:1, :].partition_broadcast(12))
            # standardize straight into the padded f32r conv-input tile
            nc.vector.tensor_scalar(out=Xri, in0=Xi,
                                    scalar1=ms[:, 0:1], scalar2=ms[:, 1:2],
                                    op0=mybir.AluOpType.subtract,
                                    op1=mybir.AluOpType.mult)

            # ---- b-conv: psum_u <- u_1 = 0.01*b  (0.01 folded into DPK)
            pu = [ps.tile([128, CH, J], f32, tag=f"pu{k}", name=f"pu{k}") for k in range(NCHUNK)]
            nmm = 25
            for i, (dy, dx) in enumerate(
                    (dy, dx) for dy in range(-2, 3) for dx in range(-2, 3)):
                for k in range(NCHUNK):
                    rv = Xr[:, PAD + dy + k * CH: PAD + dy + k * CH + CH,
                            PAD + dx: PAD + dx + J]
                    nc.tensor.matmul(pu[k][:], dt_[:, i, :], rv,
                                     start=(i == 0), stop=(i == nmm - 1))

            # ---- SBUF state
            b01 = sb.tile([128, J, J], f32r, tag="b01", name="b01")
            C = sb.tile([128, J, J], f32r, tag="C", name="C")
            A = sb.tile([128, JP, JP], f32r, tag="A", name="A")
            nc.sync.dma_start(out=A[:], in_=ZR[:])
            for k in range(NCHUNK):
                nc.scalar.activation(b01[:, k * CH:(k + 1) * CH, :], pu[k][:],
                                     mybir.ActivationFunctionType.Copy)

            Ai = A[:, PAD:PAD + J, PAD:PAD + J]

            # ---- LCA iterations (u stays in PSUM)
            for it in range(iters_dev):
                for k in range(NCHUNK):
                    cv = C[:, k * CH:(k + 1) * CH, :]
                    nc.vector.tensor_scalar(out=cv, in0=pu[k][:],
                                            scalar1=THRESH, scalar2=-THRESH,
                                            op0=mybir.AluOpType.min,
                                            op1=mybir.AluOpType.max)
                    nc.vector.tensor_sub(Ai[:, k * CH:(k + 1) * CH, :],
                                         pu[k][:], cv)
                # accumulate u += b01 - 0.01*c - 0.01*conv(a); chunk-major so
                # chunk-0 elementwise overlaps chunk-1 matmuls next iteration
                for k in range(NCHUNK):
                    nc.tensor.matmul(pu[k][:], ipk[:, 0:128],
                                     b01[:, k * CH:(k + 1) * CH, :],
                                     start=False, stop=False)
                    nc.tensor.matmul(pu[k][:], ipk[:, 128:256],
                                     C[:, k * CH:(k + 1) * CH, :],
                                     start=False, stop=False)
                    for i, (dy, dx) in enumerate(
                            (dy, dx) for dy in range(-3, 4) for dx in range(-3, 4)):
                        rv = A[:, PAD + dy + k * CH: PAD + dy + k * CH + CH,
                               PAD + dx: PAD + dx + J]
                        nc.tensor.matmul(pu[k][:], gt[:, i, :], rv,
                                         start=False, stop=(i == 48))

            # ---- final a_10 = u_9 - clip(u_9)
            aout = sb.tile([128, J, J], f32, tag="aout", name="aout")
            for k in range(NCHUNK):
                cv = C[:, k * CH:(k + 1) * CH, :]
                nc.vector.tensor_scalar(out=cv, in0=pu[k][:],
                                        scalar1=THRESH, scalar2=-THRESH,
                                        op0=mybir.AluOpType.min,
                                        op1=mybir.AluOpType.max)
                nc.vector.tensor_sub(aout[:, k * CH:(k + 1) * CH, :],
                                     pu[k][:], cv)
            nc.sync.dma_start(out=AOUT[:], in_=aout[:])

    nc.finalize()
    return nc


# ---------------------------------------------------------------- interface
def kernel(x, D, _trace=False, _iters_dev=ITERS_DEV):
    from concourse.bass_utils import run_bass_kernel_spmd

    x = np.asarray(x, np.float32)
    D = np.asarray(D, np.float32)

    key = ("nc", _iters_dev)
    if key not in _CACHE:
        _CACHE[key] = _build(_iters_dev)
    nc = _CACHE[key]

    wk = ("wts", D.tobytes()[:64])
    if "wts" not in _CACHE or _CACHE.get("wts_id") != wk:
        _CACHE["wts"] = _host_pack(D)
        _CACHE["wts_id"] = wk
    wts = _CACHE["wts"]

    core_ids = list(range(B))
    in_maps = []
    for b in range(B):
        in_maps.append({
            "XL2": _pack_x(x[b, :, 0]),
            "GPK": wts["GPK"],
            "DPK": wts["DPK"],
            "IPK": wts["IPK"],
            "ZR": _ZEROS,
            "ONESD": np.ones((12, 1), np.float32),
        })

    if _trace:
        _install_ntff_hook()
    res = run_bass_kernel_spmd(nc, in_maps, core_ids, trace=_trace)

    out = np.empty((B, NN, 1, H, W), np.float32)
    for b in range(B):
        out[b, :, 0] = _unpack_a(res.results[b]["AOUT"])
    if _trace:
        kernel._last_exec_ns = res.exec_time_ns
    return out

